# revision 1
# baseline (speedup 1.0000x reference)
import os
import sys

import numpy as np

for _p in ("/opt/trn_rl_repo", "/root/.axon_site/_ro/trn_rl_repo"):
    if os.path.isdir(_p) and _p not in sys.path:
        sys.path.insert(0, _p)

B, N, D, KD = 32, 2048, 512, 256
F = D + KD
NCORES = 8
BL = B // NCORES
NT = 16

_BUILD_CACHE = {}
last_results = None


def _build(stream_f32r: bool):
    import concourse.bass as bass
    import concourse.tile as tile
    from concourse import bacc, mybir
    from concourse.masks import make_identity

    f32 = mybir.dt.float32
    i32 = mybir.dt.int32
    mm_dt = mybir.dt.float32r if stream_f32r else f32

    nc = bacc.Bacc()

    xK_f = nc.dram_tensor("xK_f", [BL, N, D], mm_dt, kind="ExternalInput")
    xk1_f = nc.dram_tensor("xk1_f", [BL, N, KD], mm_dt, kind="ExternalInput")
    xK_b = nc.dram_tensor("xK_b", [BL, N, D], mm_dt, kind="ExternalInput")
    xk1_b = nc.dram_tensor("xk1_b", [BL, N, KD], mm_dt, kind="ExternalInput")
    adj_f = nc.dram_tensor("adj_f", [BL, N], i32, kind="ExternalInput")
    sm_f = nc.dram_tensor("sm_f", [BL, N], i32, kind="ExternalInput")
    adj_b = nc.dram_tensor("adj_b", [BL, N], i32, kind="ExternalInput")
    sm_b = nc.dram_tensor("sm_b", [BL, N], i32, kind="ExternalInput")
    v_f = nc.dram_tensor("v_f", [F], f32, kind="ExternalInput")
    v_b = nc.dram_tensor("v_b", [F], f32, kind="ExternalInput")
    G0_f = nc.dram_tensor("G0_f", [F + 1, D], f32, kind="ExternalInput")
    G1_f = nc.dram_tensor("G1_f", [F + 1, D], f32, kind="ExternalInput")
    G0_b = nc.dram_tensor("G0_b", [F + 1, D], f32, kind="ExternalInput")
    G1_b = nc.dram_tensor("G1_b", [F + 1, D], f32, kind="ExternalInput")
    out_f = nc.dram_tensor("out_f", [BL, D], f32, kind="ExternalOutput")
    out_b = nc.dram_tensor("out_b", [BL, D], f32, kind="ExternalOutput")

    branches = [
        dict(xK=xK_f, xk1=xk1_f, adj=adj_f, sm=sm_f, v=v_f, G0=G0_f, G1=G1_f, out=out_f),
        dict(xK=xK_b, xk1=xk1_b, adj=adj_b, sm=sm_b, v=v_b, G0=G0_b, G1=G1_b, out=out_b),
    ]

    with tile.TileContext(nc) as tc:
        with (
            tc.tile_pool(name="singles", bufs=1) as singles,
            tc.tile_pool(name="xKp", bufs=2) as xKp,
            tc.tile_pool(name="xk1p", bufs=3) as xk1p,
            tc.tile_pool(name="scr", bufs=3) as scr,
            tc.tile_pool(name="small", bufs=4) as small,
            tc.tile_pool(name="uallp", bufs=2) as uallp,
            tc.tile_pool(name="uallTp", bufs=2) as uallTp,
            tc.tile_pool(name="finp", bufs=2) as finp,
            tc.tile_pool(name="psU_K", bufs=2, space="PSUM") as psU_K,
            tc.tile_pool(name="psU_1", bufs=2, space="PSUM") as psU_1,
            tc.tile_pool(name="psTr", bufs=2, space="PSUM") as psTr,
            tc.tile_pool(name="psOut", bufs=1, space="PSUM") as psOut,
        ):
            ident = singles.tile([128, 128], f32)
            make_identity(nc, ident)
            ones11 = singles.tile([1, 1], f32)
            nc.vector.memset(ones11, 1.0)
            zf = singles.tile([128, NT, 8], f32)
            nc.vector.memset(zf, 0.0)
            ones2 = singles.tile([128, 2], mm_dt)
            nc.vector.tensor_scalar_add(out=ones2, in0=zf[:, 0, 0:2], scalar1=1.0)

            per_br = []
            for br in branches:
                st = {}
                vb = singles.tile([128, F], f32)
                vap = br["v"][:]
                nc.gpsimd.dma_start(
                    out=vb,
                    in_=bass.AP(tensor=vap.tensor, offset=vap.offset, ap=[[0, 128]] + vap.ap),
                )
                st["vb"] = vb
                for gname in ("G0", "G1"):
                    g = br[gname]
                    gs = singles.tile([128, 7, D], f32)
                    nc.gpsimd.dma_start(
                        out=gs[:, 0:6, :],
                        in_=g[0:F, :].rearrange("(k p) n -> p k n", p=128),
                    )
                    nc.gpsimd.dma_start(out=gs[0:1, 6, :], in_=g[F : F + 1, :])
                    st[gname] = gs
                adj_i = small.tile([128, BL, NT], i32, tag="mask_i")
                sm_i = small.tile([128, BL, NT], i32, tag="mask_i")
                nc.gpsimd.dma_start(out=adj_i, in_=br["adj"].rearrange("b (p n) -> p b n", n=NT))
                nc.gpsimd.dma_start(out=sm_i, in_=br["sm"].rearrange("b (p n) -> p b n", n=NT))
                adjf = small.tile([128, BL, NT], f32, tag="mask_f")
                smf = small.tile([128, BL, NT], f32, tag="mask_f")
                nc.vector.tensor_copy(adjf, adj_i)
                nc.vector.tensor_copy(smf, sm_i)
                m0 = singles.tile([128, BL, NT], f32, tag=f"m0_{br['out'].name}")
                m1 = singles.tile([128, BL, NT], f32, tag=f"m1_{br['out'].name}")
                nc.vector.tensor_mul(m0, adjf, smf)
                nc.vector.tensor_sub(m1, adjf, m0)
                st["m0"], st["m1"] = m0, m1
                per_br.append(st)

            for bi, br in enumerate(branches):
                st = per_br[bi]
                psK = psU_K.tile([8, D], f32)
                ps1 = psU_1.tile([8, KD + 2], f32)

                for b in range(BL):
                    xK = xKp.tile([128, NT, D], mm_dt, tag="xK")
                    nc.gpsimd.dma_start(
                        out=xK, in_=br["xK"][b].rearrange("(p n) d -> p n d", n=NT)
                    )
                    xk1 = xk1p.tile([128, NT, KD], mm_dt, tag="xk1")
                    nc.gpsimd.dma_start(
                        out=xk1, in_=br["xk1"][b].rearrange("(p n) d -> p n d", n=NT)
                    )
                    xK_f32 = xK[:, :, :].bitcast(f32)
                    xk1_f32 = xk1[:, :, :].bitcast(f32)

                    sA = small.tile([128, NT], f32, tag="sA")
                    sB = small.tile([128, NT], f32, tag="sB")
                    prodK = scr.tile([128, D], f32, tag="prodK")
                    prod1 = scr.tile([128, KD], f32, tag="prod1")
                    for n in range(NT):
                        nc.vector.scalar_tensor_tensor(
                            out=prodK,
                            in0=xK_f32[:, n, :],
                            scalar=0.0,
                            in1=st["vb"][:, 0:D],
                            op0=mybir.AluOpType.bypass,
                            op1=mybir.AluOpType.mult,
                            accum_out=sA[:, n : n + 1],
                        )
                        nc.vector.scalar_tensor_tensor(
                            out=prod1,
                            in0=xk1_f32[:, n, :],
                            scalar=0.0,
                            in1=st["vb"][:, D:F],
                            op0=mybir.AluOpType.bypass,
                            op1=mybir.AluOpType.mult,
                            accum_out=sB[:, n : n + 1],
                        )
                    nc.vector.tensor_add(sB, sA, sB)
                    p_raw = small.tile([128, NT], f32, tag="p_raw")
                    nc.scalar.activation(out=p_raw, in_=sB, func=mybir.ActivationFunctionType.Exp)

                    pp = small.tile([128, NT, 8], mm_dt, tag="pp")
                    nc.vector.tensor_mul(pp, zf, zf)
                    nc.vector.tensor_mul(pp[:, :, b], p_raw, st["m0"][:, b, :])
                    nc.vector.tensor_mul(pp[:, :, 4 + b], p_raw, st["m1"][:, b, :])

                    for n in range(NT):
                        first = b == 0 and n == 0
                        last = b == BL - 1 and n == NT - 1
                        nc.tensor.matmul(psK, pp[:, n, :], xK[:, n, :], start=first, stop=last)
                        nc.tensor.matmul(
                            ps1[:, 0:KD], pp[:, n, :], xk1[:, n, :], start=first, stop=False
                        )
                        nc.tensor.matmul(
                            ps1[:, KD : KD + 2],
                            pp[:, n, :],
                            ones2,
                            start=False,
                            stop=last,
                        )

                uall = uallp.tile([8, F + 1], f32)
                nc.vector.tensor_copy(uall[:, 0:D], psK)
                nc.vector.tensor_copy(uall[:, D : F + 1], ps1[:, 0 : KD + 1])

                uallT = uallTp.tile([128, 7, 8], f32)
                for k in range(6):
                    trp = psTr.tile([128, 8], f32)
                    nc.tensor.transpose(trp, uall[:, k * 128 : (k + 1) * 128], ident[0:8, 0:8])
                    nc.vector.tensor_copy(uallT[:, k, :], trp)
                trp = psTr.tile([128, 8], f32)
                nc.tensor.transpose(trp[0:1, :], uall[:, F : F + 1], ident[0:8, 0:8])
                nc.vector.tensor_copy(uallT[0:1, 6, :], trp[0:1, :])

                po = psOut.tile([4, D + 1], f32)
                for k in range(6):
                    nc.tensor.matmul(
                        po[:, 0:D], uallT[:, k, 0:4], st["G0"][:, k, :], start=(k == 0), stop=False
                    )
                nc.tensor.matmul(
                    po[:, 0:D], uallT[0:1, 6, 0:4], st["G0"][0:1, 6, :], start=False, stop=False
                )
                for k in range(6):
                    nc.tensor.matmul(
                        po[:, 0:D], uallT[:, k, 4:8], st["G1"][:, k, :], start=False, stop=False
                    )
                nc.tensor.matmul(
                    po[:, 0:D], uallT[0:1, 6, 4:8], st["G1"][0:1, 6, :], start=False, stop=True
                )
                nc.tensor.matmul(po[:, D : D + 1], uallT[0:1, 6, 0:4], ones11, start=True, stop=False)
                nc.tensor.matmul(po[:, D : D + 1], uallT[0:1, 6, 4:8], ones11, start=False, stop=True)

                rp = finp.tile([4, 1], f32, tag="rp")
                nc.vector.reciprocal(rp, po[:, D : D + 1])
                osb = finp.tile([4, D], f32, tag="osb")
                nc.vector.tensor_scalar_mul(out=osb, in0=po[:, 0:D], scalar1=rp)
                nc.sync.dma_start(out=br["out"][:, :], in_=osb)

    nc.compile()
    return nc


def _get_nc(stream_f32r: bool):
    key = ("nc", stream_f32r)
    if key not in _BUILD_CACHE:
        _BUILD_CACHE[key] = _build(stream_f32r)
    return _BUILD_CACHE[key]


def kernel(**inputs) -> tuple:
    global last_results
    from concourse.bass_utils import run_bass_kernel_spmd

    f32 = np.float32
    K = np.ascontiguousarray(np.asarray(inputs["K"], dtype=f32))
    front_k1 = np.ascontiguousarray(np.asarray(inputs["front_k1"], dtype=f32))
    back_K = np.ascontiguousarray(np.asarray(inputs["back_K"], dtype=f32))
    back_k2 = np.ascontiguousarray(np.asarray(inputs["back_k2"], dtype=f32))
    Wfk = np.asarray(inputs["Wfk"], dtype=f32)
    bfk = np.asarray(inputs["bfk"], dtype=f32)
    Wbk = np.asarray(inputs["Wbk"], dtype=f32)
    bbk = np.asarray(inputs["bbk"], dtype=f32)
    Wr0 = np.asarray(inputs["Wr0"], dtype=f32)
    Wr1 = np.asarray(inputs["Wr1"], dtype=f32)
    wf_den = np.asarray(inputs["wf_den"], dtype=f32)
    wb_den = np.asarray(inputs["wb_den"], dtype=f32)
    adj_f = np.ascontiguousarray(np.asarray(inputs["front_sdj_den"], dtype=np.int32))
    sm_f = np.ascontiguousarray(np.asarray(inputs["front_s_mask"], dtype=np.int32))
    adj_b = np.ascontiguousarray(np.asarray(inputs["back_sdj_den"], dtype=np.int32))
    sm_b = np.ascontiguousarray(np.asarray(inputs["back_s_mask"], dtype=np.int32))
    i = int(np.asarray(inputs["i"]))
    num_utter = int(np.asarray(inputs["num_utter"]))

    v_f = (Wfk.astype(np.float64) @ wf_den[D:].astype(np.float64)).astype(f32)
    v_b = (Wbk.astype(np.float64) @ wb_den[D:].astype(np.float64)).astype(f32)
    A_f = np.vstack([Wfk, bfk[None, :]]).astype(np.float64)
    A_b = np.vstack([Wbk, bbk[None, :]]).astype(np.float64)
    G0_f = (A_f @ Wr0.astype(np.float64)).astype(f32)
    G1_f = (A_f @ Wr1.astype(np.float64)).astype(f32)
    G0_b = (A_b @ Wr0.astype(np.float64)).astype(f32)
    G1_b = (A_b @ Wr1.astype(np.float64)).astype(f32)

    stream_f32r = os.environ.get("KERNEL_MM_F32R", "1") == "1"
    nc = _get_nc(stream_f32r)

    in_maps = []
    for c in range(NCORES):
        s = slice(c * BL, (c + 1) * BL)
        in_maps.append(
            {
                "xK_f": K[s],
                "xk1_f": front_k1[s],
                "xK_b": back_K[s],
                "xk1_b": back_k2[s],
                "adj_f": adj_f[s],
                "sm_f": sm_f[s],
                "adj_b": adj_b[s],
                "sm_b": sm_b[s],
                "v_f": v_f,
                "v_b": v_b,
                "G0_f": G0_f,
                "G1_f": G1_f,
                "G0_b": G0_b,
                "G1_b": G1_b,
            }
        )

    trace = os.environ.get("KERNEL_TRACE", "0") == "1"
    res = run_bass_kernel_spmd(nc, in_maps, core_ids=list(range(NCORES)), trace=trace)
    last_results = res

    front = np.concatenate([r["out_f"] for r in res.results], axis=0)
    back = np.concatenate([r["out_b"] for r in res.results], axis=0)
    if i == 0:
        front = np.zeros((B, D), dtype=f32)
    if i == num_utter - 1:
        back = np.zeros((B, D), dtype=f32)
    return (front, back)



# revision 2
# speedup vs baseline: 1.2412x; 1.2412x over previous
import os
import sys

import numpy as np

for _p in ("/opt/trn_rl_repo", "/root/.axon_site/_ro/trn_rl_repo"):
    if os.path.isdir(_p) and _p not in sys.path:
        sys.path.insert(0, _p)

B, N, D, KD = 32, 2048, 512, 256
F = D + KD
FP = F + 4
NCORES = 8
BL = B // NCORES
NT = 16

_BUILD_CACHE = {}
last_results = None


def _build():
    import concourse.bass as bass
    import concourse.tile as tile
    from concourse import bacc, mybir
    from concourse.masks import make_identity

    f32 = mybir.dt.float32
    f16 = mybir.dt.float16

    nc = bacc.Bacc()

    x_f = nc.dram_tensor("x_f", [BL, N, FP], f16, kind="ExternalInput")
    x_b = nc.dram_tensor("x_b", [BL, N, FP], f16, kind="ExternalInput")
    m0_f = nc.dram_tensor("m0_f", [BL, N], f16, kind="ExternalInput")
    m1_f = nc.dram_tensor("m1_f", [BL, N], f16, kind="ExternalInput")
    m0_b = nc.dram_tensor("m0_b", [BL, N], f16, kind="ExternalInput")
    m1_b = nc.dram_tensor("m1_b", [BL, N], f16, kind="ExternalInput")
    v_f = nc.dram_tensor("v_f", [FP], f16, kind="ExternalInput")
    v_b = nc.dram_tensor("v_b", [FP], f16, kind="ExternalInput")
    G0_f = nc.dram_tensor("G0_f", [F + 1, D], f16, kind="ExternalInput")
    G1_f = nc.dram_tensor("G1_f", [F + 1, D], f16, kind="ExternalInput")
    G0_b = nc.dram_tensor("G0_b", [F + 1, D], f16, kind="ExternalInput")
    G1_b = nc.dram_tensor("G1_b", [F + 1, D], f16, kind="ExternalInput")
    out_f = nc.dram_tensor("out_f", [BL, D], f32, kind="ExternalOutput")
    out_b = nc.dram_tensor("out_b", [BL, D], f32, kind="ExternalOutput")

    branches = [
        dict(x=x_f, m0=m0_f, m1=m1_f, v=v_f, G0=G0_f, G1=G1_f, out=out_f),
        dict(x=x_b, m0=m0_b, m1=m1_b, v=v_b, G0=G0_b, G1=G1_b, out=out_b),
    ]

    with tile.TileContext(nc) as tc:
        with (
            tc.tile_pool(name="singles", bufs=1) as singles,
            tc.tile_pool(name="xp", bufs=4) as xp,
            tc.tile_pool(name="scr", bufs=3) as scr,
            tc.tile_pool(name="small", bufs=4) as small,
            tc.tile_pool(name="ppp", bufs=3) as ppp,
            tc.tile_pool(name="uallp", bufs=2) as uallp,
            tc.tile_pool(name="uallTp", bufs=2) as uallTp,
            tc.tile_pool(name="finp", bufs=2) as finp,
            tc.tile_pool(name="psU_K", bufs=2, space="PSUM") as psU_K,
            tc.tile_pool(name="psU_1", bufs=2, space="PSUM") as psU_1,
            tc.tile_pool(name="psTr", bufs=2, space="PSUM") as psTr,
            tc.tile_pool(name="psOut", bufs=1, space="PSUM") as psOut,
        ):
            ident = singles.tile([128, 128], f32)
            make_identity(nc, ident)
            ones11 = singles.tile([1, 1], f32)
            nc.vector.memset(ones11, 1.0)

            per_br = []
            for br in branches:
                st = {}
                vb = singles.tile([128, FP], f16)
                vap = br["v"][:]
                nc.sync.dma_start(
                    out=vb,
                    in_=bass.AP(tensor=vap.tensor, offset=vap.offset, ap=[[0, 128]] + vap.ap),
                )
                st["vb"] = vb
                for gname in ("G0", "G1"):
                    g = br[gname]
                    gs = singles.tile([128, 7, D], f16)
                    nc.sync.dma_start(
                        out=gs[:, 0:6, :],
                        in_=g[0:F, :].rearrange("(k p) n -> p k n", p=128),
                    )
                    nc.sync.dma_start(out=gs[0:1, 6, :], in_=g[F : F + 1, :])
                    st[gname] = gs
                m0 = singles.tile([128, BL, NT], f16, tag=f"m0_{br['out'].name}")
                m1 = singles.tile([128, BL, NT], f16, tag=f"m1_{br['out'].name}")
                nc.sync.dma_start(out=m0, in_=br["m0"].rearrange("b (p n) -> p b n", n=NT))
                nc.sync.dma_start(out=m1, in_=br["m1"].rearrange("b (p n) -> p b n", n=NT))
                st["m0"], st["m1"] = m0, m1
                per_br.append(st)

            for bi, br in enumerate(branches):
                st = per_br[bi]
                psK = psU_K.tile([8, D], f32)
                ps1 = psU_1.tile([8, KD + 4], f32)

                for b in range(BL):
                    xt = xp.tile([128, NT, FP], f16, tag="xt")
                    nc.gpsimd.dma_start(
                        out=xt, in_=br["x"][b].rearrange("(p n) d -> p n d", n=NT)
                    )

                    sB = small.tile([128, NT], f32, tag="sB")
                    prod = scr.tile([128, FP], f16, tag="prod")
                    for n in range(NT):
                        nc.vector.scalar_tensor_tensor(
                            out=prod,
                            in0=xt[:, n, :],
                            scalar=0.0,
                            in1=st["vb"],
                            op0=mybir.AluOpType.bypass,
                            op1=mybir.AluOpType.mult,
                            accum_out=sB[:, n : n + 1],
                        )
                    p_raw = small.tile([128, NT], f16, tag="p_raw")
                    nc.scalar.activation(out=p_raw, in_=sB, func=mybir.ActivationFunctionType.Exp)

                    pp = ppp.tile([128, NT, 8], f16, tag="pp")
                    nc.vector.memset(pp, 0.0)
                    nc.vector.tensor_mul(pp[:, :, b], p_raw, st["m0"][:, b, :])
                    nc.vector.tensor_mul(pp[:, :, 4 + b], p_raw, st["m1"][:, b, :])

                    for n in range(NT):
                        first = b == 0 and n == 0
                        last = b == BL - 1 and n == NT - 1
                        nc.tensor.matmul(
                            psK, pp[:, n, :], xt[:, n, 0:D], start=first, stop=last
                        )
                        nc.tensor.matmul(
                            ps1, pp[:, n, :], xt[:, n, D:FP], start=first, stop=last
                        )

                uall = uallp.tile([8, F + 1], f32)
                nc.vector.tensor_copy(uall[:, 0:D], psK)
                nc.vector.tensor_copy(uall[:, D : F + 1], ps1[:, 0 : KD + 1])

                uallT = uallTp.tile([128, 7, 8], f16)
                for k in range(6):
                    trp = psTr.tile([128, 8], f32)
                    nc.tensor.transpose(trp, uall[:, k * 128 : (k + 1) * 128], ident[0:8, 0:8])
                    nc.vector.tensor_copy(uallT[:, k, :], trp)
                trp = psTr.tile([128, 8], f32)
                nc.tensor.transpose(trp[0:1, :], uall[:, F : F + 1], ident[0:8, 0:8])
                nc.vector.tensor_copy(uallT[0:1, 6, :], trp[0:1, :])
                pT = finp.tile([1, 8], f32, tag="pT")
                nc.vector.tensor_copy(pT, trp[0:1, :])

                po = psOut.tile([4, D + 1], f32)
                for k in range(6):
                    nc.tensor.matmul(
                        po[:, 0:D], uallT[:, k, 0:4], st["G0"][:, k, :], start=(k == 0), stop=False
                    )
                nc.tensor.matmul(
                    po[:, 0:D], uallT[0:1, 6, 0:4], st["G0"][0:1, 6, :], start=False, stop=False
                )
                for k in range(6):
                    nc.tensor.matmul(
                        po[:, 0:D], uallT[:, k, 4:8], st["G1"][:, k, :], start=False, stop=False
                    )
                nc.tensor.matmul(
                    po[:, 0:D], uallT[0:1, 6, 4:8], st["G1"][0:1, 6, :], start=False, stop=True
                )
                nc.tensor.matmul(po[:, D : D + 1], pT[:, 0:4], ones11, start=True, stop=False)
                nc.tensor.matmul(po[:, D : D + 1], pT[:, 4:8], ones11, start=False, stop=True)

                rp = finp.tile([4, 1], f32, tag="rp")
                nc.vector.reciprocal(rp, po[:, D : D + 1])
                osb = finp.tile([4, D], f32, tag="osb")
                nc.vector.tensor_scalar_mul(out=osb, in0=po[:, 0:D], scalar1=rp)
                nc.sync.dma_start(out=br["out"][:, :], in_=osb)

    nc.compile()
    return nc


def _get_nc():
    if "nc" not in _BUILD_CACHE:
        _BUILD_CACHE["nc"] = _build()
    return _BUILD_CACHE["nc"]


def _pack_x(Kv, k1):
    x = np.empty((B, N, FP), np.float16)
    x[:, :, 0:D] = Kv
    x[:, :, D:F] = k1
    x[:, :, F : F + 2] = 1.0
    x[:, :, F + 2 : FP] = 0.0
    return x


def kernel(**inputs) -> tuple:
    global last_results
    from concourse.bass_utils import run_bass_kernel_spmd

    f32 = np.float32
    f16 = np.float16
    Wfk = np.asarray(inputs["Wfk"], dtype=f32)
    bfk = np.asarray(inputs["bfk"], dtype=f32)
    Wbk = np.asarray(inputs["Wbk"], dtype=f32)
    bbk = np.asarray(inputs["bbk"], dtype=f32)
    Wr0 = np.asarray(inputs["Wr0"], dtype=f32)
    Wr1 = np.asarray(inputs["Wr1"], dtype=f32)
    wf_den = np.asarray(inputs["wf_den"], dtype=f32)
    wb_den = np.asarray(inputs["wb_den"], dtype=f32)
    i = int(np.asarray(inputs["i"]))
    num_utter = int(np.asarray(inputs["num_utter"]))

    x_f = _pack_x(np.asarray(inputs["K"]), np.asarray(inputs["front_k1"]))
    x_b = _pack_x(np.asarray(inputs["back_K"]), np.asarray(inputs["back_k2"]))

    adj_f = np.asarray(inputs["front_sdj_den"], dtype=f32)
    sm_f = np.asarray(inputs["front_s_mask"], dtype=f32)
    adj_b = np.asarray(inputs["back_sdj_den"], dtype=f32)
    sm_b = np.asarray(inputs["back_s_mask"], dtype=f32)
    m0_f = (adj_f * sm_f).astype(f16)
    m1_f = (adj_f * (1.0 - sm_f)).astype(f16)
    m0_b = (adj_b * sm_b).astype(f16)
    m1_b = (adj_b * (1.0 - sm_b)).astype(f16)

    def fold_v(Wk, wden):
        v = np.zeros((FP,), f16)
        v[0:F] = (Wk.astype(np.float64) @ wden[D:].astype(np.float64)).astype(f16)
        return v

    v_f = fold_v(Wfk, wf_den)
    v_b = fold_v(Wbk, wb_den)
    A_f = np.vstack([Wfk, bfk[None, :]]).astype(np.float64)
    A_b = np.vstack([Wbk, bbk[None, :]]).astype(np.float64)
    G0_f = (A_f @ Wr0.astype(np.float64)).astype(f16)
    G1_f = (A_f @ Wr1.astype(np.float64)).astype(f16)
    G0_b = (A_b @ Wr0.astype(np.float64)).astype(f16)
    G1_b = (A_b @ Wr1.astype(np.float64)).astype(f16)

    nc = _get_nc()

    in_maps = []
    for c in range(NCORES):
        s = slice(c * BL, (c + 1) * BL)
        in_maps.append(
            {
                "x_f": x_f[s],
                "x_b": x_b[s],
                "m0_f": m0_f[s],
                "m1_f": m1_f[s],
                "m0_b": m0_b[s],
                "m1_b": m1_b[s],
                "v_f": v_f,
                "v_b": v_b,
                "G0_f": G0_f,
                "G1_f": G1_f,
                "G0_b": G0_b,
                "G1_b": G1_b,
            }
        )

    trace = os.environ.get("KERNEL_TRACE", "0") == "1"
    res = run_bass_kernel_spmd(nc, in_maps, core_ids=list(range(NCORES)), trace=trace)
    last_results = res

    front = np.concatenate([r["out_f"] for r in res.results], axis=0)
    back = np.concatenate([r["out_b"] for r in res.results], axis=0)
    if i == 0:
        front = np.zeros((B, D), dtype=f32)
    if i == num_utter - 1:
        back = np.zeros((B, D), dtype=f32)
    return (front, back)


# revision 5
# speedup vs baseline: 1.3352x; 1.0758x over previous
import os
import sys

import numpy as np

for _p in ("/opt/trn_rl_repo", "/root/.axon_site/_ro/trn_rl_repo"):
    if os.path.isdir(_p) and _p not in sys.path:
        sys.path.insert(0, _p)

B, N, D, KD = 32, 2048, 512, 256
F = D + KD
FP = F + 4
NCORES = 8
BL = B // NCORES
NT = 16
NEGM = -70.0

_BUILD_CACHE = {}
last_results = None


def _build():
    import concourse.bass as bass
    import concourse.tile as tile
    from concourse import bacc, mybir
    from concourse.masks import make_identity

    f32 = mybir.dt.float32
    f16 = mybir.dt.float16

    nc = bacc.Bacc()

    x_f = nc.dram_tensor("x_f", [BL, N, FP], f16, kind="ExternalInput")
    x_b = nc.dram_tensor("x_b", [BL, N, FP], f16, kind="ExternalInput")
    m0_f = nc.dram_tensor("m0_f", [BL, N], f16, kind="ExternalInput")
    m1_f = nc.dram_tensor("m1_f", [BL, N], f16, kind="ExternalInput")
    m0_b = nc.dram_tensor("m0_b", [BL, N], f16, kind="ExternalInput")
    m1_b = nc.dram_tensor("m1_b", [BL, N], f16, kind="ExternalInput")
    v_f = nc.dram_tensor("v_f", [FP], f16, kind="ExternalInput")
    v_b = nc.dram_tensor("v_b", [FP], f16, kind="ExternalInput")
    G0_f = nc.dram_tensor("G0_f", [F + 1, D], f16, kind="ExternalInput")
    G1_f = nc.dram_tensor("G1_f", [F + 1, D], f16, kind="ExternalInput")
    G0_b = nc.dram_tensor("G0_b", [F + 1, D], f16, kind="ExternalInput")
    G1_b = nc.dram_tensor("G1_b", [F + 1, D], f16, kind="ExternalInput")
    out_f = nc.dram_tensor("out_f", [BL, D], f32, kind="ExternalOutput")
    out_b = nc.dram_tensor("out_b", [BL, D], f32, kind="ExternalOutput")

    branches = [
        dict(x=x_f, m0=m0_f, m1=m1_f, v=v_f, G0=G0_f, G1=G1_f, out=out_f),
        dict(x=x_b, m0=m0_b, m1=m1_b, v=v_b, G0=G0_b, G1=G1_b, out=out_b),
    ]

    with tile.TileContext(nc) as tc:
        with (
            tc.tile_pool(name="singles", bufs=1) as singles,
            tc.tile_pool(name="xp", bufs=3) as xp,
            tc.tile_pool(name="prodp", bufs=2) as prodp,
            tc.tile_pool(name="scr", bufs=3) as scr,
            tc.tile_pool(name="small", bufs=4) as small,
            tc.tile_pool(name="ppp", bufs=3) as ppp,
            tc.tile_pool(name="uallp", bufs=2) as uallp,
            tc.tile_pool(name="uallTp", bufs=2) as uallTp,
            tc.tile_pool(name="finp", bufs=2) as finp,
            tc.tile_pool(name="psU_K", bufs=2, space="PSUM") as psU_K,
            tc.tile_pool(name="psU_1", bufs=2, space="PSUM") as psU_1,
            tc.tile_pool(name="psTr", bufs=2, space="PSUM") as psTr,
            tc.tile_pool(name="psOut", bufs=1, space="PSUM") as psOut,
        ):
            ident = singles.tile([128, 128], f32)
            make_identity(nc, ident)
            ones11 = singles.tile([1, 1], f32)
            nc.vector.memset(ones11, 1.0)

            per_br = []
            for br in branches:
                st = {}
                vb = singles.tile([128, FP], f16)
                vap = br["v"][:]
                nc.sync.dma_start(
                    out=vb,
                    in_=bass.AP(tensor=vap.tensor, offset=vap.offset, ap=[[0, 128]] + vap.ap),
                )
                st["vb"] = vb
                for gname in ("G0", "G1"):
                    g = br[gname]
                    gs = singles.tile([128, 7, D], f16)
                    nc.sync.dma_start(
                        out=gs[:, 0:6, :],
                        in_=g[0:F, :].rearrange("(k p) n -> p k n", p=128),
                    )
                    nc.sync.dma_start(out=gs[0:1, 6, :], in_=g[F : F + 1, :])
                    st[gname] = gs
                m0 = singles.tile([128, BL, NT], f16, tag=f"m0_{br['out'].name}")
                m1 = singles.tile([128, BL, NT], f16, tag=f"m1_{br['out'].name}")
                nc.sync.dma_start(out=m0, in_=br["m0"].rearrange("b (p n) -> p b n", n=NT))
                nc.sync.dma_start(out=m1, in_=br["m1"].rearrange("b (p n) -> p b n", n=NT))
                st["m0"], st["m1"] = m0, m1
                per_br.append(st)

            NU = 2 * BL
            state = {}

            def stage_a(u):
                bi, b = divmod(u, BL)
                br, st = branches[bi], per_br[bi]
                k = 7 if u % 2 == 0 else 8
                m = NT - k
                xt = xp.tile([128, NT, FP], f16, tag="xt")
                nc.gpsimd.dma_start(
                    out=xt, in_=br["x"][b].rearrange("(p n) d -> p n d", n=NT)
                )
                pp = ppp.tile([128, NT, 8], f16, tag="pp")
                nc.vector.memset(pp, 0.0)
                sB = small.tile([128, NT], f32, tag="sB")
                prodm = prodp.tile([128, 9, FP], f16, tag="prodm")
                vbb = bass.AP(
                    tensor=st["vb"].tensor,
                    offset=st["vb"].offset,
                    ap=[st["vb"].ap[0]] + [[0, m]] + st["vb"].ap[1:],
                )
                nc.vector.tensor_mul(prodm[:, 0:m, :], xt[:, 0:m, :], vbb)
                for j in range(m):
                    nc.scalar.activation(
                        out=prodm[:, j, :],
                        in_=prodm[:, j, :],
                        func=mybir.ActivationFunctionType.Copy,
                        accum_out=sB[:, j : j + 1],
                    )
                prod = scr.tile([128, FP], f16, tag="prod")
                for n in range(m, NT):
                    nc.vector.scalar_tensor_tensor(
                        out=prod,
                        in0=xt[:, n, :],
                        scalar=0.0,
                        in1=st["vb"],
                        op0=mybir.AluOpType.bypass,
                        op1=mybir.AluOpType.mult,
                        accum_out=sB[:, n : n + 1],
                    )
                state[u] = (xt, pp, sB)

            def stage_b(u):
                bi, b = divmod(u, BL)
                br, st = branches[bi], per_br[bi]
                xt, pp, sB = state.pop(u)
                s0 = small.tile([128, NT], f16, tag="s0")
                s1 = small.tile([128, NT], f16, tag="s1")
                nc.vector.tensor_add(s0, sB, st["m0"][:, b, :])
                nc.vector.tensor_add(s1, sB, st["m1"][:, b, :])
                nc.scalar.activation(
                    out=pp[:, :, b], in_=s0, func=mybir.ActivationFunctionType.Exp
                )
                nc.scalar.activation(
                    out=pp[:, :, 4 + b], in_=s1, func=mybir.ActivationFunctionType.Exp
                )
                psK, ps1 = state["ps", bi]
                for n in range(NT):
                    first = b == 0 and n == 0
                    last = b == BL - 1 and n == NT - 1
                    nc.tensor.matmul(
                        psK, pp[:, n, :], xt[:, n, 0:D], start=first, stop=last
                    )
                    nc.tensor.matmul(
                        ps1, pp[:, n, :], xt[:, n, D:FP], start=first, stop=last
                    )

            def finishing(bi):
                br, st = branches[bi], per_br[bi]
                psK, ps1 = state.pop(("ps", bi))
                uall = uallp.tile([8, F + 1], f32)
                nc.vector.tensor_copy(uall[:, 0:D], psK)
                nc.vector.tensor_copy(uall[:, D : F + 1], ps1[:, 0 : KD + 1])

                uallT = uallTp.tile([128, 7, 8], f16)
                for k in range(6):
                    trp = psTr.tile([128, 8], f32)
                    nc.tensor.transpose(trp, uall[:, k * 128 : (k + 1) * 128], ident[0:8, 0:8])
                    nc.vector.tensor_copy(uallT[:, k, :], trp)
                trp = psTr.tile([128, 8], f32)
                nc.tensor.transpose(trp[0:1, :], uall[:, F : F + 1], ident[0:8, 0:8])
                nc.vector.tensor_copy(uallT[0:1, 6, :], trp[0:1, :])
                pT = finp.tile([1, 8], f32, tag="pT")
                nc.vector.tensor_copy(pT, trp[0:1, :])

                po = psOut.tile([4, D + 1], f32)
                for k in range(6):
                    nc.tensor.matmul(
                        po[:, 0:D], uallT[:, k, 0:4], st["G0"][:, k, :], start=(k == 0), stop=False
                    )
                nc.tensor.matmul(
                    po[:, 0:D], uallT[0:1, 6, 0:4], st["G0"][0:1, 6, :], start=False, stop=False
                )
                for k in range(6):
                    nc.tensor.matmul(
                        po[:, 0:D], uallT[:, k, 4:8], st["G1"][:, k, :], start=False, stop=False
                    )
                nc.tensor.matmul(
                    po[:, 0:D], uallT[0:1, 6, 4:8], st["G1"][0:1, 6, :], start=False, stop=True
                )
                nc.tensor.matmul(po[:, D : D + 1], pT[:, 0:4], ones11, start=True, stop=False)
                nc.tensor.matmul(po[:, D : D + 1], pT[:, 4:8], ones11, start=False, stop=True)

                rp = finp.tile([4, 1], f32, tag="rp")
                nc.vector.reciprocal(rp, po[:, D : D + 1])
                osb = finp.tile([4, D], f32, tag="osb")
                nc.vector.tensor_scalar_mul(out=osb, in0=po[:, 0:D], scalar1=rp)
                nc.sync.dma_start(out=br["out"][:, :], in_=osb)

            for bi in range(2):
                psK = psU_K.tile([8, D], f32, tag="psK")
                ps1 = psU_1.tile([8, KD + 4], f32, tag="ps1")
                state["ps", bi] = (psK, ps1)
            for u in range(NU + 1):
                if u < NU:
                    stage_a(u)
                if u >= 1:
                    stage_b(u - 1)
                    if (u - 1) % BL == BL - 1:
                        finishing((u - 1) // BL)

    nc.compile()
    return nc


def _get_nc():
    if "nc" not in _BUILD_CACHE:
        _BUILD_CACHE["nc"] = _build()
    return _BUILD_CACHE["nc"]


def _pack_x(Kv, k1):
    x = np.empty((B, N, FP), np.float16)
    x[:, :, 0:D] = Kv
    x[:, :, D:F] = k1
    x[:, :, F : F + 2] = 1.0
    x[:, :, F + 2 : FP] = 0.0
    return x


def kernel(**inputs) -> tuple:
    global last_results
    from concourse.bass_utils import run_bass_kernel_spmd

    f32 = np.float32
    f16 = np.float16
    Wfk = np.asarray(inputs["Wfk"], dtype=f32)
    bfk = np.asarray(inputs["bfk"], dtype=f32)
    Wbk = np.asarray(inputs["Wbk"], dtype=f32)
    bbk = np.asarray(inputs["bbk"], dtype=f32)
    Wr0 = np.asarray(inputs["Wr0"], dtype=f32)
    Wr1 = np.asarray(inputs["Wr1"], dtype=f32)
    wf_den = np.asarray(inputs["wf_den"], dtype=f32)
    wb_den = np.asarray(inputs["wb_den"], dtype=f32)
    i = int(np.asarray(inputs["i"]))
    num_utter = int(np.asarray(inputs["num_utter"]))

    x_f = _pack_x(np.asarray(inputs["K"]), np.asarray(inputs["front_k1"]))
    x_b = _pack_x(np.asarray(inputs["back_K"]), np.asarray(inputs["back_k2"]))

    adj_f = np.asarray(inputs["front_sdj_den"], dtype=f32)
    sm_f = np.asarray(inputs["front_s_mask"], dtype=f32)
    adj_b = np.asarray(inputs["back_sdj_den"], dtype=f32)
    sm_b = np.asarray(inputs["back_s_mask"], dtype=f32)
    m0_f = (NEGM * (1.0 - adj_f * sm_f)).astype(f16)
    m1_f = (NEGM * (1.0 - adj_f * (1.0 - sm_f))).astype(f16)
    m0_b = (NEGM * (1.0 - adj_b * sm_b)).astype(f16)
    m1_b = (NEGM * (1.0 - adj_b * (1.0 - sm_b))).astype(f16)

    def fold_v(Wk, wden):
        v = np.zeros((FP,), f16)
        v[0:F] = (Wk.astype(np.float64) @ wden[D:].astype(np.float64)).astype(f16)
        return v

    v_f = fold_v(Wfk, wf_den)
    v_b = fold_v(Wbk, wb_den)
    A_f = np.vstack([Wfk, bfk[None, :]]).astype(np.float64)
    A_b = np.vstack([Wbk, bbk[None, :]]).astype(np.float64)
    G0_f = (A_f @ Wr0.astype(np.float64)).astype(f16)
    G1_f = (A_f @ Wr1.astype(np.float64)).astype(f16)
    G0_b = (A_b @ Wr0.astype(np.float64)).astype(f16)
    G1_b = (A_b @ Wr1.astype(np.float64)).astype(f16)

    nc = _get_nc()

    in_maps = []
    for c in range(NCORES):
        s = slice(c * BL, (c + 1) * BL)
        in_maps.append(
            {
                "x_f": x_f[s],
                "x_b": x_b[s],
                "m0_f": m0_f[s],
                "m1_f": m1_f[s],
                "m0_b": m0_b[s],
                "m1_b": m1_b[s],
                "v_f": v_f,
                "v_b": v_b,
                "G0_f": G0_f,
                "G1_f": G1_f,
                "G0_b": G0_b,
                "G1_b": G1_b,
            }
        )

    trace = os.environ.get("KERNEL_TRACE", "0") == "1"
    res = run_bass_kernel_spmd(nc, in_maps, core_ids=list(range(NCORES)), trace=trace)
    last_results = res

    front = np.concatenate([r["out_f"] for r in res.results], axis=0)
    back = np.concatenate([r["out_b"] for r in res.results], axis=0)
    if i == 0:
        front = np.zeros((B, D), dtype=f32)
    if i == num_utter - 1:
        back = np.zeros((B, D), dtype=f32)
    return (front, back)


# revision 7
# speedup vs baseline: 1.3921x; 1.0426x over previous
import os
import sys

import numpy as np

for _p in ("/opt/trn_rl_repo", "/root/.axon_site/_ro/trn_rl_repo"):
    if os.path.isdir(_p) and _p not in sys.path:
        sys.path.insert(0, _p)

B, N, D, KD = 32, 2048, 512, 256
F = D + KD
FP = F + 4
NCORES = 8
BL = B // NCORES
NT = 16
NEGM = -70.0

_BUILD_CACHE = {}
last_results = None


def _build():
    import concourse.bass as bass
    import concourse.tile as tile
    from concourse import bacc, mybir
    from concourse.masks import make_identity

    f32 = mybir.dt.float32
    f16 = mybir.dt.float16

    nc = bacc.Bacc()

    x_f = nc.dram_tensor("x_f", [BL, N, FP], f16, kind="ExternalInput")
    x_b = nc.dram_tensor("x_b", [BL, N, FP], f16, kind="ExternalInput")
    m0_f = nc.dram_tensor("m0_f", [BL, N], f16, kind="ExternalInput")
    m1_f = nc.dram_tensor("m1_f", [BL, N], f16, kind="ExternalInput")
    m0_b = nc.dram_tensor("m0_b", [BL, N], f16, kind="ExternalInput")
    m1_b = nc.dram_tensor("m1_b", [BL, N], f16, kind="ExternalInput")
    v_f = nc.dram_tensor("v_f", [FP], f16, kind="ExternalInput")
    v_b = nc.dram_tensor("v_b", [FP], f16, kind="ExternalInput")
    G0_f = nc.dram_tensor("G0_f", [F + 1, D], f16, kind="ExternalInput")
    G1_f = nc.dram_tensor("G1_f", [F + 1, D], f16, kind="ExternalInput")
    G0_b = nc.dram_tensor("G0_b", [F + 1, D], f16, kind="ExternalInput")
    G1_b = nc.dram_tensor("G1_b", [F + 1, D], f16, kind="ExternalInput")
    out_f = nc.dram_tensor("out_f", [BL, D], f32, kind="ExternalOutput")
    out_b = nc.dram_tensor("out_b", [BL, D], f32, kind="ExternalOutput")

    branches = [
        dict(x=x_f, m0=m0_f, m1=m1_f, v=v_f, G0=G0_f, G1=G1_f, out=out_f),
        dict(x=x_b, m0=m0_b, m1=m1_b, v=v_b, G0=G0_b, G1=G1_b, out=out_b),
    ]

    with tile.TileContext(nc) as tc:
        with (
            tc.tile_pool(name="singles", bufs=1) as singles,
            tc.tile_pool(name="xp", bufs=3) as xp,
            tc.tile_pool(name="prodp", bufs=3) as prodp,
            tc.tile_pool(name="scr", bufs=3) as scr,
            tc.tile_pool(name="small", bufs=4) as small,
            tc.tile_pool(name="ppp", bufs=3) as ppp,
            tc.tile_pool(name="uallp", bufs=2) as uallp,
            tc.tile_pool(name="uallTp", bufs=2) as uallTp,
            tc.tile_pool(name="finp", bufs=2) as finp,
            tc.tile_pool(name="psU_K", bufs=2, space="PSUM") as psU_K,
            tc.tile_pool(name="psU_1", bufs=2, space="PSUM") as psU_1,
            tc.tile_pool(name="psTr", bufs=2, space="PSUM") as psTr,
            tc.tile_pool(name="psOut", bufs=1, space="PSUM") as psOut,
        ):
            ident = singles.tile([128, 128], f32)
            make_identity(nc, ident)
            ones11 = singles.tile([1, 1], f32)
            nc.vector.memset(ones11, 1.0)

            per_br = []
            for br in branches:
                st = {}
                vb = singles.tile([128, FP], f16)
                vap = br["v"][:]
                nc.sync.dma_start(
                    out=vb,
                    in_=bass.AP(tensor=vap.tensor, offset=vap.offset, ap=[[0, 128]] + vap.ap),
                )
                st["vb"] = vb
                for gname in ("G0", "G1"):
                    g = br[gname]
                    gs = singles.tile([128, 7, D], f16)
                    nc.sync.dma_start(
                        out=gs[:, 0:6, :],
                        in_=g[0:F, :].rearrange("(k p) n -> p k n", p=128),
                    )
                    nc.sync.dma_start(out=gs[0:1, 6, :], in_=g[F : F + 1, :])
                    st[gname] = gs
                m0 = singles.tile([128, BL, NT], f16, tag=f"m0_{br['out'].name}")
                m1 = singles.tile([128, BL, NT], f16, tag=f"m1_{br['out'].name}")
                nc.sync.dma_start(out=m0, in_=br["m0"].rearrange("b (p n) -> p b n", n=NT))
                nc.sync.dma_start(out=m1, in_=br["m1"].rearrange("b (p n) -> p b n", n=NT))
                st["m0"], st["m1"] = m0, m1
                per_br.append(st)

            NU = 2 * BL
            state = {}

            def stage_a(u):
                bi, b = divmod(u, BL)
                br, st = branches[bi], per_br[bi]
                k = 7 if u % 2 == 0 else 8
                m = NT - k
                xt = xp.tile([128, NT, FP], f16, tag="xt")
                nc.gpsimd.dma_start(
                    out=xt, in_=br["x"][b].rearrange("(p n) d -> p n d", n=NT)
                )
                pp = ppp.tile([128, NT, 8], f16, tag="pp")
                nc.vector.memset(pp, 0.0)
                sB = small.tile([128, NT], f32, tag="sB")
                prodm = prodp.tile([128, 9, FP], f16, tag="prodm")
                vbb = bass.AP(
                    tensor=st["vb"].tensor,
                    offset=st["vb"].offset,
                    ap=[st["vb"].ap[0]] + [[0, m]] + st["vb"].ap[1:],
                )
                nc.vector.tensor_mul(prodm[:, 0:m, :], xt[:, 0:m, :], vbb)
                for j in range(m):
                    nc.scalar.activation(
                        out=prodm[:, j, :],
                        in_=prodm[:, j, :],
                        func=mybir.ActivationFunctionType.Copy,
                        accum_out=sB[:, j : j + 1],
                    )
                prod = scr.tile([128, FP], f16, tag="prod")
                for n in range(m, NT):
                    nc.vector.scalar_tensor_tensor(
                        out=prod,
                        in0=xt[:, n, :],
                        scalar=0.0,
                        in1=st["vb"],
                        op0=mybir.AluOpType.bypass,
                        op1=mybir.AluOpType.mult,
                        accum_out=sB[:, n : n + 1],
                    )
                state[u] = (xt, pp, sB)

            def stage_b(u):
                bi, b = divmod(u, BL)
                br, st = branches[bi], per_br[bi]
                xt, pp, sB = state.pop(u)
                s0 = small.tile([128, NT], f16, tag="s0")
                s1 = small.tile([128, NT], f16, tag="s1")
                nc.vector.tensor_add(s0, sB, st["m0"][:, b, :])
                nc.vector.tensor_add(s1, sB, st["m1"][:, b, :])
                nc.scalar.activation(
                    out=pp[:, :, b], in_=s0, func=mybir.ActivationFunctionType.Exp
                )
                nc.scalar.activation(
                    out=pp[:, :, 4 + b], in_=s1, func=mybir.ActivationFunctionType.Exp
                )
                psK, ps1 = state["ps", bi]
                for n in range(NT):
                    first = b == 0 and n == 0
                    last = b == BL - 1 and n == NT - 1
                    nc.tensor.matmul(
                        psK, pp[:, n, :], xt[:, n, 0:D], start=first, stop=last
                    )
                    nc.tensor.matmul(
                        ps1, pp[:, n, :], xt[:, n, D:FP], start=first, stop=last
                    )

            def finishing(bi):
                br, st = branches[bi], per_br[bi]
                psK, ps1 = state.pop(("ps", bi))
                uall = uallp.tile([8, F + 1], f32)
                nc.vector.tensor_copy(uall[:, 0:D], psK)
                nc.vector.tensor_copy(uall[:, D : F + 1], ps1[:, 0 : KD + 1])

                uallT = uallTp.tile([128, 7, 8], f16)
                for k in range(6):
                    trp = psTr.tile([128, 8], f32)
                    nc.tensor.transpose(trp, uall[:, k * 128 : (k + 1) * 128], ident[0:8, 0:8])
                    nc.vector.tensor_copy(uallT[:, k, :], trp)
                trp = psTr.tile([128, 8], f32)
                nc.tensor.transpose(trp[0:1, :], uall[:, F : F + 1], ident[0:8, 0:8])
                nc.vector.tensor_copy(uallT[0:1, 6, :], trp[0:1, :])
                pT = finp.tile([1, 8], f32, tag="pT")
                nc.vector.tensor_copy(pT, trp[0:1, :])

                po = psOut.tile([4, D + 1], f32)
                for k in range(6):
                    nc.tensor.matmul(
                        po[:, 0:D], uallT[:, k, 0:4], st["G0"][:, k, :], start=(k == 0), stop=False
                    )
                nc.tensor.matmul(
                    po[:, 0:D], uallT[0:1, 6, 0:4], st["G0"][0:1, 6, :], start=False, stop=False
                )
                for k in range(6):
                    nc.tensor.matmul(
                        po[:, 0:D], uallT[:, k, 4:8], st["G1"][:, k, :], start=False, stop=False
                    )
                nc.tensor.matmul(
                    po[:, 0:D], uallT[0:1, 6, 4:8], st["G1"][0:1, 6, :], start=False, stop=True
                )
                nc.tensor.matmul(po[:, D : D + 1], pT[:, 0:4], ones11, start=True, stop=False)
                nc.tensor.matmul(po[:, D : D + 1], pT[:, 4:8], ones11, start=False, stop=True)

                rp = finp.tile([4, 1], f32, tag="rp")
                nc.vector.reciprocal(rp, po[:, D : D + 1])
                osb = finp.tile([4, D], f32, tag="osb")
                nc.vector.tensor_scalar_mul(out=osb, in0=po[:, 0:D], scalar1=rp)
                nc.sync.dma_start(out=br["out"][:, :], in_=osb)

            for bi in range(2):
                psK = psU_K.tile([8, D], f32, tag="psK")
                ps1 = psU_1.tile([8, KD + 4], f32, tag="ps1")
                state["ps", bi] = (psK, ps1)
            for u in range(NU + 1):
                if u >= 1:
                    stage_b(u - 1)
                if u < NU:
                    stage_a(u)
                if u >= 1 and (u - 1) % BL == BL - 1:
                    finishing((u - 1) // BL)

    nc.compile()
    return nc


def _get_nc():
    if "nc" not in _BUILD_CACHE:
        _BUILD_CACHE["nc"] = _build()
    return _BUILD_CACHE["nc"]


def _pack_x(Kv, k1):
    x = np.empty((B, N, FP), np.float16)
    x[:, :, 0:D] = Kv
    x[:, :, D:F] = k1
    x[:, :, F : F + 2] = 1.0
    x[:, :, F + 2 : FP] = 0.0
    return x


def kernel(**inputs) -> tuple:
    global last_results
    from concourse.bass_utils import run_bass_kernel_spmd

    f32 = np.float32
    f16 = np.float16
    Wfk = np.asarray(inputs["Wfk"], dtype=f32)
    bfk = np.asarray(inputs["bfk"], dtype=f32)
    Wbk = np.asarray(inputs["Wbk"], dtype=f32)
    bbk = np.asarray(inputs["bbk"], dtype=f32)
    Wr0 = np.asarray(inputs["Wr0"], dtype=f32)
    Wr1 = np.asarray(inputs["Wr1"], dtype=f32)
    wf_den = np.asarray(inputs["wf_den"], dtype=f32)
    wb_den = np.asarray(inputs["wb_den"], dtype=f32)
    i = int(np.asarray(inputs["i"]))
    num_utter = int(np.asarray(inputs["num_utter"]))

    x_f = _pack_x(np.asarray(inputs["K"]), np.asarray(inputs["front_k1"]))
    x_b = _pack_x(np.asarray(inputs["back_K"]), np.asarray(inputs["back_k2"]))

    adj_f = np.asarray(inputs["front_sdj_den"], dtype=f32)
    sm_f = np.asarray(inputs["front_s_mask"], dtype=f32)
    adj_b = np.asarray(inputs["back_sdj_den"], dtype=f32)
    sm_b = np.asarray(inputs["back_s_mask"], dtype=f32)
    m0_f = (NEGM * (1.0 - adj_f * sm_f)).astype(f16)
    m1_f = (NEGM * (1.0 - adj_f * (1.0 - sm_f))).astype(f16)
    m0_b = (NEGM * (1.0 - adj_b * sm_b)).astype(f16)
    m1_b = (NEGM * (1.0 - adj_b * (1.0 - sm_b))).astype(f16)

    def fold_v(Wk, wden):
        v = np.zeros((FP,), f16)
        v[0:F] = (Wk.astype(np.float64) @ wden[D:].astype(np.float64)).astype(f16)
        return v

    v_f = fold_v(Wfk, wf_den)
    v_b = fold_v(Wbk, wb_den)
    A_f = np.vstack([Wfk, bfk[None, :]]).astype(np.float64)
    A_b = np.vstack([Wbk, bbk[None, :]]).astype(np.float64)
    G0_f = (A_f @ Wr0.astype(np.float64)).astype(f16)
    G1_f = (A_f @ Wr1.astype(np.float64)).astype(f16)
    G0_b = (A_b @ Wr0.astype(np.float64)).astype(f16)
    G1_b = (A_b @ Wr1.astype(np.float64)).astype(f16)

    nc = _get_nc()

    in_maps = []
    for c in range(NCORES):
        s = slice(c * BL, (c + 1) * BL)
        in_maps.append(
            {
                "x_f": x_f[s],
                "x_b": x_b[s],
                "m0_f": m0_f[s],
                "m1_f": m1_f[s],
                "m0_b": m0_b[s],
                "m1_b": m1_b[s],
                "v_f": v_f,
                "v_b": v_b,
                "G0_f": G0_f,
                "G1_f": G1_f,
                "G0_b": G0_b,
                "G1_b": G1_b,
            }
        )

    trace = os.environ.get("KERNEL_TRACE", "0") == "1"
    res = run_bass_kernel_spmd(nc, in_maps, core_ids=list(range(NCORES)), trace=trace)
    last_results = res

    front = np.concatenate([r["out_f"] for r in res.results], axis=0)
    back = np.concatenate([r["out_b"] for r in res.results], axis=0)
    if i == 0:
        front = np.zeros((B, D), dtype=f32)
    if i == num_utter - 1:
        back = np.zeros((B, D), dtype=f32)
    return (front, back)


# revision 9
# speedup vs baseline: 1.4578x; 1.0472x over previous
import os
import sys

import numpy as np

for _p in ("/opt/trn_rl_repo", "/root/.axon_site/_ro/trn_rl_repo"):
    if os.path.isdir(_p) and _p not in sys.path:
        sys.path.insert(0, _p)

B, N, D, KD = 32, 2048, 512, 256
F = D + KD
FP = F + 4
NCORES = 8
BL = B // NCORES
NT = 16
NEGM = -70.0

_BUILD_CACHE = {}
last_results = None


def _build():
    import concourse.bass as bass
    import concourse.tile as tile
    from concourse import bacc, mybir
    from concourse.masks import make_identity

    f32 = mybir.dt.float32
    f16 = mybir.dt.float16

    nc = bacc.Bacc()

    x_f = nc.dram_tensor("x_f", [BL, N, FP], f16, kind="ExternalInput")
    x_b = nc.dram_tensor("x_b", [BL, N, FP], f16, kind="ExternalInput")
    m0_f = nc.dram_tensor("m0_f", [BL, N], f16, kind="ExternalInput")
    m1_f = nc.dram_tensor("m1_f", [BL, N], f16, kind="ExternalInput")
    m0_b = nc.dram_tensor("m0_b", [BL, N], f16, kind="ExternalInput")
    m1_b = nc.dram_tensor("m1_b", [BL, N], f16, kind="ExternalInput")
    v_f = nc.dram_tensor("v_f", [FP], f16, kind="ExternalInput")
    v_b = nc.dram_tensor("v_b", [FP], f16, kind="ExternalInput")
    G0_f = nc.dram_tensor("G0_f", [F + 1, D], f16, kind="ExternalInput")
    G1_f = nc.dram_tensor("G1_f", [F + 1, D], f16, kind="ExternalInput")
    G0_b = nc.dram_tensor("G0_b", [F + 1, D], f16, kind="ExternalInput")
    G1_b = nc.dram_tensor("G1_b", [F + 1, D], f16, kind="ExternalInput")
    out_f = nc.dram_tensor("out_f", [BL, D], f32, kind="ExternalOutput")
    out_b = nc.dram_tensor("out_b", [BL, D], f32, kind="ExternalOutput")

    branches = [
        dict(x=x_f, m0=m0_f, m1=m1_f, v=v_f, G0=G0_f, G1=G1_f, out=out_f),
        dict(x=x_b, m0=m0_b, m1=m1_b, v=v_b, G0=G0_b, G1=G1_b, out=out_b),
    ]

    with tile.TileContext(nc) as tc:
        with (
            tc.tile_pool(name="singles", bufs=1) as singles,
            tc.tile_pool(name="xp", bufs=3) as xp,
            tc.tile_pool(name="prodp", bufs=3) as prodp,
            tc.tile_pool(name="scr", bufs=3) as scr,
            tc.tile_pool(name="small", bufs=4) as small,
            tc.tile_pool(name="ppp", bufs=3) as ppp,
            tc.tile_pool(name="uallp", bufs=2) as uallp,
            tc.tile_pool(name="uallTp", bufs=2) as uallTp,
            tc.tile_pool(name="finp", bufs=2) as finp,
            tc.tile_pool(name="psU_K", bufs=2, space="PSUM") as psU_K,
            tc.tile_pool(name="psU_1", bufs=2, space="PSUM") as psU_1,
            tc.tile_pool(name="psTr", bufs=2, space="PSUM") as psTr,
            tc.tile_pool(name="psOut", bufs=1, space="PSUM") as psOut,
        ):
            ident = singles.tile([128, 128], f32)
            make_identity(nc, ident)
            ones11 = singles.tile([1, 1], f32)
            nc.vector.memset(ones11, 1.0)

            per_br = []
            for br in branches:
                st = {}
                m0 = singles.tile([128, BL, NT], f16, tag=f"m0_{br['out'].name}")
                m1 = singles.tile([128, BL, NT], f16, tag=f"m1_{br['out'].name}")
                nc.sync.dma_start(out=m0, in_=br["m0"].rearrange("b (p n) -> p b n", n=NT))
                nc.sync.dma_start(out=m1, in_=br["m1"].rearrange("b (p n) -> p b n", n=NT))
                st["m0"], st["m1"] = m0, m1
                vb = singles.tile([128, FP], f16, tag=f"vb_{br['out'].name}")
                vap = br["v"][:]
                nc.sync.dma_start(
                    out=vb,
                    in_=bass.AP(tensor=vap.tensor, offset=vap.offset, ap=[[0, 128]] + vap.ap),
                )
                st["vb"] = vb
                per_br.append(st)

            def load_g(bi):
                br, st = branches[bi], per_br[bi]
                for gname in ("G0", "G1"):
                    g = br[gname]
                    gs = singles.tile([128, 7, D], f16, tag=f"{gname}_{bi}")
                    nc.sync.dma_start(
                        out=gs[:, 0:6, :],
                        in_=g[0:F, :].rearrange("(k p) n -> p k n", p=128),
                    )
                    nc.sync.dma_start(out=gs[0:1, 6, :], in_=g[F : F + 1, :])
                    st[gname] = gs

            NU = 2 * BL
            state = {}

            def stage_a(u):
                bi, b = divmod(u, BL)
                br, st = branches[bi], per_br[bi]
                k = 7 if u % 2 == 0 else 8
                m = NT - k
                xt = xp.tile([128, NT, FP], f16, tag="xt")
                nc.gpsimd.dma_start(
                    out=xt, in_=br["x"][b].rearrange("(p n) d -> p n d", n=NT)
                )
                pp = ppp.tile([128, NT, 8], f16, tag="pp")
                nc.vector.memset(pp, 0.0)
                sB = small.tile([128, NT], f32, tag="sB")
                prodm = prodp.tile([128, 9, FP], f16, tag="prodm")
                vbb = bass.AP(
                    tensor=st["vb"].tensor,
                    offset=st["vb"].offset,
                    ap=[st["vb"].ap[0]] + [[0, m]] + st["vb"].ap[1:],
                )
                nc.vector.tensor_mul(prodm[:, 0:m, :], xt[:, 0:m, :], vbb)
                for j in range(m):
                    nc.scalar.activation(
                        out=prodm[:, j, :],
                        in_=prodm[:, j, :],
                        func=mybir.ActivationFunctionType.Copy,
                        accum_out=sB[:, j : j + 1],
                    )
                prod = scr.tile([128, FP], f16, tag="prod")
                for n in range(m, NT):
                    nc.vector.scalar_tensor_tensor(
                        out=prod,
                        in0=xt[:, n, :],
                        scalar=0.0,
                        in1=st["vb"],
                        op0=mybir.AluOpType.bypass,
                        op1=mybir.AluOpType.mult,
                        accum_out=sB[:, n : n + 1],
                    )
                state[u] = (xt, pp, sB)

            def stage_b(u):
                bi, b = divmod(u, BL)
                br, st = branches[bi], per_br[bi]
                xt, pp, sB = state.pop(u)
                s0 = small.tile([128, NT], f16, tag="s0")
                s1 = small.tile([128, NT], f16, tag="s1")
                nc.vector.tensor_add(s0, sB, st["m0"][:, b, :])
                nc.vector.tensor_add(s1, sB, st["m1"][:, b, :])
                nc.scalar.activation(
                    out=pp[:, :, b], in_=s0, func=mybir.ActivationFunctionType.Exp
                )
                nc.scalar.activation(
                    out=pp[:, :, 4 + b], in_=s1, func=mybir.ActivationFunctionType.Exp
                )
                psK, ps1 = state["ps", bi]
                for n in range(NT):
                    first = b == 0 and n == 0
                    last = b == BL - 1 and n == NT - 1
                    nc.tensor.matmul(
                        psK, pp[:, n, :], xt[:, n, 0:D], start=first, stop=last
                    )
                    nc.tensor.matmul(
                        ps1, pp[:, n, :], xt[:, n, D:FP], start=first, stop=last
                    )

            def finishing(bi):
                br, st = branches[bi], per_br[bi]
                psK, ps1 = state.pop(("ps", bi))
                uall = uallp.tile([8, F + 1], f32)
                nc.vector.tensor_copy(uall[:, 0:D], psK)
                nc.vector.tensor_copy(uall[:, D : F + 1], ps1[:, 0 : KD + 1])

                uallT = uallTp.tile([128, 7, 8], f16)
                for k in range(6):
                    trp = psTr.tile([128, 8], f32)
                    nc.tensor.transpose(trp, uall[:, k * 128 : (k + 1) * 128], ident[0:8, 0:8])
                    nc.vector.tensor_copy(uallT[:, k, :], trp)
                trp = psTr.tile([128, 8], f32)
                nc.tensor.transpose(trp[0:1, :], uall[:, F : F + 1], ident[0:8, 0:8])
                nc.vector.tensor_copy(uallT[0:1, 6, :], trp[0:1, :])
                pT = finp.tile([1, 8], f32, tag="pT")
                nc.vector.tensor_copy(pT, trp[0:1, :])

                po = psOut.tile([4, D + 1], f32)
                for k in range(6):
                    nc.tensor.matmul(
                        po[:, 0:D], uallT[:, k, 0:4], st["G0"][:, k, :], start=(k == 0), stop=False
                    )
                nc.tensor.matmul(
                    po[:, 0:D], uallT[0:1, 6, 0:4], st["G0"][0:1, 6, :], start=False, stop=False
                )
                for k in range(6):
                    nc.tensor.matmul(
                        po[:, 0:D], uallT[:, k, 4:8], st["G1"][:, k, :], start=False, stop=False
                    )
                nc.tensor.matmul(
                    po[:, 0:D], uallT[0:1, 6, 4:8], st["G1"][0:1, 6, :], start=False, stop=True
                )
                nc.tensor.matmul(po[:, D : D + 1], pT[:, 0:4], ones11, start=True, stop=False)
                nc.tensor.matmul(po[:, D : D + 1], pT[:, 4:8], ones11, start=False, stop=True)

                rp = finp.tile([4, 1], f32, tag="rp")
                nc.vector.reciprocal(rp, po[:, D : D + 1])
                osb = finp.tile([4, D], f32, tag="osb")
                nc.vector.tensor_scalar_mul(out=osb, in0=po[:, 0:D], scalar1=rp)
                nc.sync.dma_start(out=br["out"][:, :], in_=osb)

            for bi in range(2):
                psK = psU_K.tile([8, D], f32, tag="psK")
                ps1 = psU_1.tile([8, KD + 4], f32, tag="ps1")
                state["ps", bi] = (psK, ps1)
            for u in range(NU + 1):
                if u >= 1:
                    stage_b(u - 1)
                if u < NU:
                    stage_a(u)
                if u == 2:
                    load_g(0)
                if u == BL + 2:
                    load_g(1)
                if u >= 1 and (u - 1) % BL == BL - 1:
                    finishing((u - 1) // BL)

    nc.compile()
    return nc


def _get_nc():
    if "nc" not in _BUILD_CACHE:
        _BUILD_CACHE["nc"] = _build()
    return _BUILD_CACHE["nc"]


def _pack_x(Kv, k1):
    x = np.empty((B, N, FP), np.float16)
    x[:, :, 0:D] = Kv
    x[:, :, D:F] = k1
    x[:, :, F : F + 2] = 1.0
    x[:, :, F + 2 : FP] = 0.0
    return x


def kernel(**inputs) -> tuple:
    global last_results
    from concourse.bass_utils import run_bass_kernel_spmd

    f32 = np.float32
    f16 = np.float16
    Wfk = np.asarray(inputs["Wfk"], dtype=f32)
    bfk = np.asarray(inputs["bfk"], dtype=f32)
    Wbk = np.asarray(inputs["Wbk"], dtype=f32)
    bbk = np.asarray(inputs["bbk"], dtype=f32)
    Wr0 = np.asarray(inputs["Wr0"], dtype=f32)
    Wr1 = np.asarray(inputs["Wr1"], dtype=f32)
    wf_den = np.asarray(inputs["wf_den"], dtype=f32)
    wb_den = np.asarray(inputs["wb_den"], dtype=f32)
    i = int(np.asarray(inputs["i"]))
    num_utter = int(np.asarray(inputs["num_utter"]))

    x_f = _pack_x(np.asarray(inputs["K"]), np.asarray(inputs["front_k1"]))
    x_b = _pack_x(np.asarray(inputs["back_K"]), np.asarray(inputs["back_k2"]))

    adj_f = np.asarray(inputs["front_sdj_den"], dtype=f32)
    sm_f = np.asarray(inputs["front_s_mask"], dtype=f32)
    adj_b = np.asarray(inputs["back_sdj_den"], dtype=f32)
    sm_b = np.asarray(inputs["back_s_mask"], dtype=f32)
    m0_f = (NEGM * (1.0 - adj_f * sm_f)).astype(f16)
    m1_f = (NEGM * (1.0 - adj_f * (1.0 - sm_f))).astype(f16)
    m0_b = (NEGM * (1.0 - adj_b * sm_b)).astype(f16)
    m1_b = (NEGM * (1.0 - adj_b * (1.0 - sm_b))).astype(f16)

    def fold_v(Wk, wden):
        v = np.zeros((FP,), f16)
        v[0:F] = (Wk.astype(np.float64) @ wden[D:].astype(np.float64)).astype(f16)
        return v

    v_f = fold_v(Wfk, wf_den)
    v_b = fold_v(Wbk, wb_den)
    A_f = np.vstack([Wfk, bfk[None, :]]).astype(np.float64)
    A_b = np.vstack([Wbk, bbk[None, :]]).astype(np.float64)
    G0_f = (A_f @ Wr0.astype(np.float64)).astype(f16)
    G1_f = (A_f @ Wr1.astype(np.float64)).astype(f16)
    G0_b = (A_b @ Wr0.astype(np.float64)).astype(f16)
    G1_b = (A_b @ Wr1.astype(np.float64)).astype(f16)

    nc = _get_nc()

    in_maps = []
    for c in range(NCORES):
        s = slice(c * BL, (c + 1) * BL)
        in_maps.append(
            {
                "x_f": x_f[s],
                "x_b": x_b[s],
                "m0_f": m0_f[s],
                "m1_f": m1_f[s],
                "m0_b": m0_b[s],
                "m1_b": m1_b[s],
                "v_f": v_f,
                "v_b": v_b,
                "G0_f": G0_f,
                "G1_f": G1_f,
                "G0_b": G0_b,
                "G1_b": G1_b,
            }
        )

    trace = os.environ.get("KERNEL_TRACE", "0") == "1"
    res = run_bass_kernel_spmd(nc, in_maps, core_ids=list(range(NCORES)), trace=trace)
    last_results = res

    front = np.concatenate([r["out_f"] for r in res.results], axis=0)
    back = np.concatenate([r["out_b"] for r in res.results], axis=0)
    if i == 0:
        front = np.zeros((B, D), dtype=f32)
    if i == num_utter - 1:
        back = np.zeros((B, D), dtype=f32)
    return (front, back)


# revision 10
# speedup vs baseline: 1.6488x; 1.1310x over previous
import os
import sys

import numpy as np

for _p in ("/opt/trn_rl_repo", "/root/.axon_site/_ro/trn_rl_repo"):
    if os.path.isdir(_p) and _p not in sys.path:
        sys.path.insert(0, _p)

B, N, D, KD = 32, 2048, 512, 256
F = D + KD
FP = F + 4
NCORES = 8
BL = B // NCORES
NT = 16
NEGM = -70.0

_BUILD_CACHE = {}
last_results = None


def _build():
    import concourse.bass as bass
    import concourse.tile as tile
    from concourse import bacc, mybir
    from concourse.masks import make_identity

    f32 = mybir.dt.float32
    f16 = mybir.dt.float16

    nc = bacc.Bacc()

    x_f = nc.dram_tensor("x_f", [BL, N, FP], f16, kind="ExternalInput")
    x_b = nc.dram_tensor("x_b", [BL, N, FP], f16, kind="ExternalInput")
    m0_f = nc.dram_tensor("m0_f", [128, BL, NT], f16, kind="ExternalInput")
    m1_f = nc.dram_tensor("m1_f", [128, BL, NT], f16, kind="ExternalInput")
    m0_b = nc.dram_tensor("m0_b", [128, BL, NT], f16, kind="ExternalInput")
    m1_b = nc.dram_tensor("m1_b", [128, BL, NT], f16, kind="ExternalInput")
    v_f = nc.dram_tensor("v_f", [128, FP], f16, kind="ExternalInput")
    v_b = nc.dram_tensor("v_b", [128, FP], f16, kind="ExternalInput")
    G0_f = nc.dram_tensor("G0_f", [128, 7, D], f16, kind="ExternalInput")
    G1_f = nc.dram_tensor("G1_f", [128, 7, D], f16, kind="ExternalInput")
    G0_b = nc.dram_tensor("G0_b", [128, 7, D], f16, kind="ExternalInput")
    G1_b = nc.dram_tensor("G1_b", [128, 7, D], f16, kind="ExternalInput")
    out_f = nc.dram_tensor("out_f", [BL, D], f32, kind="ExternalOutput")
    out_b = nc.dram_tensor("out_b", [BL, D], f32, kind="ExternalOutput")

    branches = [
        dict(x=x_f, m0=m0_f, m1=m1_f, v=v_f, G0=G0_f, G1=G1_f, out=out_f),
        dict(x=x_b, m0=m0_b, m1=m1_b, v=v_b, G0=G0_b, G1=G1_b, out=out_b),
    ]

    with tile.TileContext(nc) as tc:
        with (
            tc.tile_pool(name="singles", bufs=1) as singles,
            tc.tile_pool(name="xp", bufs=4) as xp,
            tc.tile_pool(name="prodp", bufs=3) as prodp,
            tc.tile_pool(name="scr", bufs=3) as scr,
            tc.tile_pool(name="small", bufs=4) as small,
            tc.tile_pool(name="ppp", bufs=3) as ppp,
            tc.tile_pool(name="uallp", bufs=2) as uallp,
            tc.tile_pool(name="uallTp", bufs=2) as uallTp,
            tc.tile_pool(name="finp", bufs=2) as finp,
            tc.tile_pool(name="psU_K", bufs=2, space="PSUM") as psU_K,
            tc.tile_pool(name="psU_1", bufs=2, space="PSUM") as psU_1,
            tc.tile_pool(name="psTr", bufs=2, space="PSUM") as psTr,
            tc.tile_pool(name="psOut", bufs=1, space="PSUM") as psOut,
        ):
            ident = singles.tile([128, 128], f32)
            make_identity(nc, ident)
            ones11 = singles.tile([1, 1], f32)
            nc.vector.memset(ones11, 1.0)

            per_br = []
            for br in branches:
                st = {}
                m0 = singles.tile([128, BL, NT], f16, tag=f"m0_{br['out'].name}")
                m1 = singles.tile([128, BL, NT], f16, tag=f"m1_{br['out'].name}")
                nc.sync.dma_start(out=m0, in_=br["m0"][:, :, :])
                nc.sync.dma_start(out=m1, in_=br["m1"][:, :, :])
                st["m0"], st["m1"] = m0, m1
                vb = singles.tile([128, FP], f16, tag=f"vb_{br['out'].name}")
                nc.sync.dma_start(out=vb, in_=br["v"][:, :])
                st["vb"] = vb
                per_br.append(st)

            def load_g(bi):
                br, st = branches[bi], per_br[bi]
                for gname in ("G0", "G1"):
                    g = br[gname]
                    gs = singles.tile([128, 7, D], f16, tag=f"{gname}_{bi}")
                    nc.gpsimd.dma_start(out=gs, in_=g[:, :, :])
                    st[gname] = gs

            NU = 2 * BL
            state = {}

            def stage_a(u):
                bi, b = divmod(u, BL)
                br, st = branches[bi], per_br[bi]
                k = 7 if u % 2 == 0 else 8
                m = NT - k
                xt = xp.tile([128, NT, FP], f16, tag="xt")
                nc.gpsimd.dma_start(
                    out=xt, in_=br["x"][b].rearrange("(p n) d -> p n d", n=NT)
                )
                pp = ppp.tile([128, NT, 8], f16, tag="pp")
                nc.vector.memset(pp, 0.0)
                sB = small.tile([128, NT], f32, tag="sB")
                prodm = prodp.tile([128, 9, FP], f16, tag="prodm")
                vbb = bass.AP(
                    tensor=st["vb"].tensor,
                    offset=st["vb"].offset,
                    ap=[st["vb"].ap[0]] + [[0, m]] + st["vb"].ap[1:],
                )
                nc.vector.tensor_mul(prodm[:, 0:m, :], xt[:, 0:m, :], vbb)
                for j in range(m):
                    nc.scalar.activation(
                        out=prodm[:, j, :],
                        in_=prodm[:, j, :],
                        func=mybir.ActivationFunctionType.Copy,
                        accum_out=sB[:, j : j + 1],
                    )
                prod = scr.tile([128, FP], f16, tag="prod")
                for n in range(m, NT):
                    nc.vector.scalar_tensor_tensor(
                        out=prod,
                        in0=xt[:, n, :],
                        scalar=0.0,
                        in1=st["vb"],
                        op0=mybir.AluOpType.bypass,
                        op1=mybir.AluOpType.mult,
                        accum_out=sB[:, n : n + 1],
                    )
                state[u] = (xt, pp, sB)

            def stage_b(u):
                bi, b = divmod(u, BL)
                br, st = branches[bi], per_br[bi]
                xt, pp, sB = state.pop(u)
                s0 = small.tile([128, NT], f16, tag="s0")
                s1 = small.tile([128, NT], f16, tag="s1")
                nc.vector.tensor_add(s0, sB, st["m0"][:, b, :])
                nc.vector.tensor_add(s1, sB, st["m1"][:, b, :])
                nc.scalar.activation(
                    out=pp[:, :, b], in_=s0, func=mybir.ActivationFunctionType.Exp
                )
                nc.scalar.activation(
                    out=pp[:, :, 4 + b], in_=s1, func=mybir.ActivationFunctionType.Exp
                )
                psK, ps1 = state["ps", bi]
                for n in range(NT):
                    first = b == 0 and n == 0
                    last = b == BL - 1 and n == NT - 1
                    nc.tensor.matmul(
                        psK, pp[:, n, :], xt[:, n, 0:D], start=first, stop=last
                    )
                    nc.tensor.matmul(
                        ps1, pp[:, n, :], xt[:, n, D:FP], start=first, stop=last
                    )

            def finishing(bi):
                br, st = branches[bi], per_br[bi]
                psK, ps1 = state.pop(("ps", bi))
                uall = uallp.tile([8, F + 1], f32)
                nc.vector.tensor_copy(uall[:, 0:D], psK)
                nc.vector.tensor_copy(uall[:, D : F + 1], ps1[:, 0 : KD + 1])

                uallT = uallTp.tile([128, 7, 8], f16)
                for k in range(6):
                    trp = psTr.tile([128, 8], f32)
                    nc.tensor.transpose(trp, uall[:, k * 128 : (k + 1) * 128], ident[0:8, 0:8])
                    nc.vector.tensor_copy(uallT[:, k, :], trp)
                trp = psTr.tile([128, 8], f32)
                nc.tensor.transpose(trp[0:1, :], uall[:, F : F + 1], ident[0:8, 0:8])
                nc.vector.tensor_copy(uallT[0:1, 6, :], trp[0:1, :])
                pT = finp.tile([1, 8], f32, tag="pT")
                nc.vector.tensor_copy(pT, trp[0:1, :])

                po = psOut.tile([4, D + 1], f32)
                for k in range(6):
                    nc.tensor.matmul(
                        po[:, 0:D], uallT[:, k, 0:4], st["G0"][:, k, :], start=(k == 0), stop=False
                    )
                nc.tensor.matmul(
                    po[:, 0:D], uallT[0:1, 6, 0:4], st["G0"][0:1, 6, :], start=False, stop=False
                )
                for k in range(6):
                    nc.tensor.matmul(
                        po[:, 0:D], uallT[:, k, 4:8], st["G1"][:, k, :], start=False, stop=False
                    )
                nc.tensor.matmul(
                    po[:, 0:D], uallT[0:1, 6, 4:8], st["G1"][0:1, 6, :], start=False, stop=True
                )
                nc.tensor.matmul(po[:, D : D + 1], pT[:, 0:4], ones11, start=True, stop=False)
                nc.tensor.matmul(po[:, D : D + 1], pT[:, 4:8], ones11, start=False, stop=True)

                rp = finp.tile([4, 1], f32, tag="rp")
                nc.vector.reciprocal(rp, po[:, D : D + 1])
                osb = finp.tile([4, D], f32, tag="osb")
                nc.vector.tensor_scalar_mul(out=osb, in0=po[:, 0:D], scalar1=rp)
                nc.sync.dma_start(out=br["out"][:, :], in_=osb)

            for bi in range(2):
                psK = psU_K.tile([8, D], f32, tag="psK")
                ps1 = psU_1.tile([8, KD + 4], f32, tag="ps1")
                state["ps", bi] = (psK, ps1)
            for u in range(NU + 1):
                if u >= 1:
                    stage_b(u - 1)
                if u < NU:
                    stage_a(u)
                if u == 2:
                    load_g(0)
                if u == BL + 1:
                    load_g(1)
                if u >= 1 and (u - 1) % BL == BL - 1:
                    finishing((u - 1) // BL)

    nc.compile()
    return nc


def _get_nc():
    if "nc" not in _BUILD_CACHE:
        _BUILD_CACHE["nc"] = _build()
    return _BUILD_CACHE["nc"]


def _pack_x(Kv, k1):
    x = np.empty((B, N, FP), np.float16)
    x[:, :, 0:D] = Kv
    x[:, :, D:F] = k1
    x[:, :, F : F + 2] = 1.0
    x[:, :, F + 2 : FP] = 0.0
    return x


def kernel(**inputs) -> tuple:
    global last_results
    from concourse.bass_utils import run_bass_kernel_spmd

    f32 = np.float32
    f16 = np.float16
    Wfk = np.asarray(inputs["Wfk"], dtype=f32)
    bfk = np.asarray(inputs["bfk"], dtype=f32)
    Wbk = np.asarray(inputs["Wbk"], dtype=f32)
    bbk = np.asarray(inputs["bbk"], dtype=f32)
    Wr0 = np.asarray(inputs["Wr0"], dtype=f32)
    Wr1 = np.asarray(inputs["Wr1"], dtype=f32)
    wf_den = np.asarray(inputs["wf_den"], dtype=f32)
    wb_den = np.asarray(inputs["wb_den"], dtype=f32)
    i = int(np.asarray(inputs["i"]))
    num_utter = int(np.asarray(inputs["num_utter"]))

    x_f = _pack_x(np.asarray(inputs["K"]), np.asarray(inputs["front_k1"]))
    x_b = _pack_x(np.asarray(inputs["back_K"]), np.asarray(inputs["back_k2"]))

    adj_f = np.asarray(inputs["front_sdj_den"], dtype=f32)
    sm_f = np.asarray(inputs["front_s_mask"], dtype=f32)
    adj_b = np.asarray(inputs["back_sdj_den"], dtype=f32)
    sm_b = np.asarray(inputs["back_s_mask"], dtype=f32)

    def pack_mask(m):
        m = (NEGM * (1.0 - m)).astype(f16)
        m = m.reshape(NCORES, BL, 128, NT)
        return np.ascontiguousarray(m.transpose(0, 2, 1, 3))

    m0_f = pack_mask(adj_f * sm_f)
    m1_f = pack_mask(adj_f * (1.0 - sm_f))
    m0_b = pack_mask(adj_b * sm_b)
    m1_b = pack_mask(adj_b * (1.0 - sm_b))

    def fold_v(Wk, wden):
        v = np.zeros((FP,), f16)
        v[0:F] = (Wk.astype(np.float64) @ wden[D:].astype(np.float64)).astype(f16)
        return np.ascontiguousarray(np.broadcast_to(v, (128, FP)))

    v_f = fold_v(Wfk, wf_den)
    v_b = fold_v(Wbk, wb_den)
    A_f = np.vstack([Wfk, bfk[None, :]]).astype(np.float64)
    A_b = np.vstack([Wbk, bbk[None, :]]).astype(np.float64)
    def pack_g(G):
        gs = np.zeros((128, 7, D), f16)
        gs[:, 0:6, :] = G[0:F].reshape(6, 128, D).transpose(1, 0, 2)
        gs[0, 6, :] = G[F]
        return gs

    G0_f = pack_g((A_f @ Wr0.astype(np.float64)).astype(f16))
    G1_f = pack_g((A_f @ Wr1.astype(np.float64)).astype(f16))
    G0_b = pack_g((A_b @ Wr0.astype(np.float64)).astype(f16))
    G1_b = pack_g((A_b @ Wr1.astype(np.float64)).astype(f16))

    nc = _get_nc()

    in_maps = []
    for c in range(NCORES):
        s = slice(c * BL, (c + 1) * BL)
        in_maps.append(
            {
                "x_f": x_f[s],
                "x_b": x_b[s],
                "m0_f": m0_f[c],
                "m1_f": m1_f[c],
                "m0_b": m0_b[c],
                "m1_b": m1_b[c],
                "v_f": v_f,
                "v_b": v_b,
                "G0_f": G0_f,
                "G1_f": G1_f,
                "G0_b": G0_b,
                "G1_b": G1_b,
            }
        )

    trace = os.environ.get("KERNEL_TRACE", "0") == "1"
    res = run_bass_kernel_spmd(nc, in_maps, core_ids=list(range(NCORES)), trace=trace)
    last_results = res

    front = np.concatenate([r["out_f"] for r in res.results], axis=0)
    back = np.concatenate([r["out_b"] for r in res.results], axis=0)
    if i == 0:
        front = np.zeros((B, D), dtype=f32)
    if i == num_utter - 1:
        back = np.zeros((B, D), dtype=f32)
    return (front, back)


# revision 11
# speedup vs baseline: 1.6996x; 1.0308x over previous
import os
import sys

import numpy as np

for _p in ("/opt/trn_rl_repo", "/root/.axon_site/_ro/trn_rl_repo"):
    if os.path.isdir(_p) and _p not in sys.path:
        sys.path.insert(0, _p)

B, N, D, KD = 32, 2048, 512, 256
F = D + KD
FP = F + 4
NCORES = 8
BL = B // NCORES
NT = 16
NEGM = -70.0

_BUILD_CACHE = {}
last_results = None


def _build():
    import concourse.bass as bass
    import concourse.tile as tile
    from concourse import bacc, mybir
    from concourse.masks import make_identity

    f32 = mybir.dt.float32
    f16 = mybir.dt.float16

    nc = bacc.Bacc()

    x_f = nc.dram_tensor("x_f", [BL, N, FP], f16, kind="ExternalInput")
    x_b = nc.dram_tensor("x_b", [BL, N, FP], f16, kind="ExternalInput")
    m0_f = nc.dram_tensor("m0_f", [128, BL, NT], f16, kind="ExternalInput")
    m1_f = nc.dram_tensor("m1_f", [128, BL, NT], f16, kind="ExternalInput")
    m0_b = nc.dram_tensor("m0_b", [128, BL, NT], f16, kind="ExternalInput")
    m1_b = nc.dram_tensor("m1_b", [128, BL, NT], f16, kind="ExternalInput")
    v_f = nc.dram_tensor("v_f", [128, FP], f16, kind="ExternalInput")
    v_b = nc.dram_tensor("v_b", [128, FP], f16, kind="ExternalInput")
    G0_f = nc.dram_tensor("G0_f", [128, 7, D], f16, kind="ExternalInput")
    G1_f = nc.dram_tensor("G1_f", [128, 7, D], f16, kind="ExternalInput")
    G0_b = nc.dram_tensor("G0_b", [128, 7, D], f16, kind="ExternalInput")
    G1_b = nc.dram_tensor("G1_b", [128, 7, D], f16, kind="ExternalInput")
    out_f = nc.dram_tensor("out_f", [BL, D], f32, kind="ExternalOutput")
    out_b = nc.dram_tensor("out_b", [BL, D], f32, kind="ExternalOutput")

    branches = [
        dict(x=x_f, m0=m0_f, m1=m1_f, v=v_f, G0=G0_f, G1=G1_f, out=out_f),
        dict(x=x_b, m0=m0_b, m1=m1_b, v=v_b, G0=G0_b, G1=G1_b, out=out_b),
    ]

    with tile.TileContext(nc) as tc:
        with (
            tc.tile_pool(name="singles", bufs=1) as singles,
            tc.tile_pool(name="xp", bufs=4) as xp,
            tc.tile_pool(name="prodp", bufs=3) as prodp,
            tc.tile_pool(name="scr", bufs=3) as scr,
            tc.tile_pool(name="small", bufs=4) as small,
            tc.tile_pool(name="ppp", bufs=3) as ppp,
            tc.tile_pool(name="uallp", bufs=2) as uallp,
            tc.tile_pool(name="uallTp", bufs=2) as uallTp,
            tc.tile_pool(name="finp", bufs=2) as finp,
            tc.tile_pool(name="psU_K", bufs=2, space="PSUM") as psU_K,
            tc.tile_pool(name="psU_1", bufs=2, space="PSUM") as psU_1,
            tc.tile_pool(name="psTr", bufs=2, space="PSUM") as psTr,
            tc.tile_pool(name="psOut", bufs=1, space="PSUM") as psOut,
        ):
            ident = singles.tile([128, 128], f32)
            make_identity(nc, ident)
            ones11 = singles.tile([1, 1], f32)
            nc.vector.memset(ones11, 1.0)

            per_br = []
            for br in branches:
                st = {}
                m0 = singles.tile([128, BL, NT], f16, tag=f"m0_{br['out'].name}")
                m1 = singles.tile([128, BL, NT], f16, tag=f"m1_{br['out'].name}")
                nc.gpsimd.dma_start(out=m0, in_=br["m0"][:, :, :])
                nc.gpsimd.dma_start(out=m1, in_=br["m1"][:, :, :])
                st["m0"], st["m1"] = m0, m1
                vb = singles.tile([128, FP], f16, tag=f"vb_{br['out'].name}")
                nc.gpsimd.dma_start(out=vb, in_=br["v"][:, :])
                st["vb"] = vb
                per_br.append(st)

            def load_g(bi):
                br, st = branches[bi], per_br[bi]
                for gname in ("G0", "G1"):
                    g = br[gname]
                    gs = singles.tile([128, 7, D], f16, tag=f"{gname}_{bi}")
                    nc.gpsimd.dma_start(out=gs, in_=g[:, :, :])
                    st[gname] = gs

            NU = 2 * BL
            state = {}

            def stage_a(u):
                bi, b = divmod(u, BL)
                br, st = branches[bi], per_br[bi]
                k = 7 if u % 2 == 0 else 8
                m = NT - k
                xt = xp.tile([128, NT, FP], f16, tag="xt")
                nc.gpsimd.dma_start(
                    out=xt, in_=br["x"][b].rearrange("(p n) d -> p n d", n=NT)
                )
                pp = ppp.tile([128, NT, 8], f16, tag="pp")
                nc.vector.memset(pp, 0.0)
                sB = small.tile([128, NT], f32, tag="sB")
                prodm = prodp.tile([128, 9, FP], f16, tag="prodm")
                vbb = bass.AP(
                    tensor=st["vb"].tensor,
                    offset=st["vb"].offset,
                    ap=[st["vb"].ap[0]] + [[0, m]] + st["vb"].ap[1:],
                )
                nc.vector.tensor_mul(prodm[:, 0:m, :], xt[:, 0:m, :], vbb)
                for j in range(m):
                    nc.scalar.activation(
                        out=prodm[:, j, :],
                        in_=prodm[:, j, :],
                        func=mybir.ActivationFunctionType.Copy,
                        accum_out=sB[:, j : j + 1],
                    )
                prod = scr.tile([128, FP], f16, tag="prod")
                for n in range(m, NT):
                    nc.vector.scalar_tensor_tensor(
                        out=prod,
                        in0=xt[:, n, :],
                        scalar=0.0,
                        in1=st["vb"],
                        op0=mybir.AluOpType.bypass,
                        op1=mybir.AluOpType.mult,
                        accum_out=sB[:, n : n + 1],
                    )
                state[u] = (xt, pp, sB)

            def stage_b(u):
                bi, b = divmod(u, BL)
                br, st = branches[bi], per_br[bi]
                xt, pp, sB = state.pop(u)
                s0 = small.tile([128, NT], f16, tag="s0")
                s1 = small.tile([128, NT], f16, tag="s1")
                nc.vector.tensor_add(s0, sB, st["m0"][:, b, :])
                nc.vector.tensor_add(s1, sB, st["m1"][:, b, :])
                nc.scalar.activation(
                    out=pp[:, :, b], in_=s0, func=mybir.ActivationFunctionType.Exp
                )
                nc.scalar.activation(
                    out=pp[:, :, 4 + b], in_=s1, func=mybir.ActivationFunctionType.Exp
                )
                psK, ps1 = state["ps", bi]
                for n in range(NT):
                    first = b == 0 and n == 0
                    last = b == BL - 1 and n == NT - 1
                    nc.tensor.matmul(
                        psK, pp[:, n, :], xt[:, n, 0:D], start=first, stop=last
                    )
                    nc.tensor.matmul(
                        ps1, pp[:, n, :], xt[:, n, D:FP], start=first, stop=last
                    )

            def finishing(bi):
                br, st = branches[bi], per_br[bi]
                psK, ps1 = state.pop(("ps", bi))
                uall = uallp.tile([8, F + 1], f32)
                nc.vector.tensor_copy(uall[:, 0:D], psK)
                nc.vector.tensor_copy(uall[:, D : F + 1], ps1[:, 0 : KD + 1])

                uallT = uallTp.tile([128, 7, 8], f16)
                for k in range(6):
                    trp = psTr.tile([128, 8], f32)
                    nc.tensor.transpose(trp, uall[:, k * 128 : (k + 1) * 128], ident[0:8, 0:8])
                    nc.vector.tensor_copy(uallT[:, k, :], trp)
                trp = psTr.tile([128, 8], f32)
                nc.tensor.transpose(trp[0:1, :], uall[:, F : F + 1], ident[0:8, 0:8])
                nc.vector.tensor_copy(uallT[0:1, 6, :], trp[0:1, :])
                pT = finp.tile([1, 8], f32, tag="pT")
                nc.vector.tensor_copy(pT, trp[0:1, :])

                po = psOut.tile([4, D + 1], f32)
                for k in range(6):
                    nc.tensor.matmul(
                        po[:, 0:D], uallT[:, k, 0:4], st["G0"][:, k, :], start=(k == 0), stop=False
                    )
                nc.tensor.matmul(
                    po[:, 0:D], uallT[0:1, 6, 0:4], st["G0"][0:1, 6, :], start=False, stop=False
                )
                for k in range(6):
                    nc.tensor.matmul(
                        po[:, 0:D], uallT[:, k, 4:8], st["G1"][:, k, :], start=False, stop=False
                    )
                nc.tensor.matmul(
                    po[:, 0:D], uallT[0:1, 6, 4:8], st["G1"][0:1, 6, :], start=False, stop=True
                )
                nc.tensor.matmul(po[:, D : D + 1], pT[:, 0:4], ones11, start=True, stop=False)
                nc.tensor.matmul(po[:, D : D + 1], pT[:, 4:8], ones11, start=False, stop=True)

                rp = finp.tile([4, 1], f32, tag="rp")
                nc.vector.reciprocal(rp, po[:, D : D + 1])
                osb = finp.tile([4, D], f32, tag="osb")
                nc.vector.tensor_scalar_mul(out=osb, in0=po[:, 0:D], scalar1=rp)
                nc.sync.dma_start(out=br["out"][:, :], in_=osb)

            for bi in range(2):
                psK = psU_K.tile([8, D], f32, tag="psK")
                ps1 = psU_1.tile([8, KD + 4], f32, tag="ps1")
                state["ps", bi] = (psK, ps1)
            for u in range(NU + 1):
                if u >= 1:
                    stage_b(u - 1)
                if u < NU:
                    stage_a(u)
                if u == 3:
                    load_g(0)
                if u == BL + 1:
                    load_g(1)
                if u >= 1 and (u - 1) % BL == BL - 1:
                    finishing((u - 1) // BL)

    nc.compile()
    return nc


def _get_nc():
    if "nc" not in _BUILD_CACHE:
        _BUILD_CACHE["nc"] = _build()
    return _BUILD_CACHE["nc"]


def _pack_x(Kv, k1):
    x = np.empty((B, N, FP), np.float16)
    x[:, :, 0:D] = Kv
    x[:, :, D:F] = k1
    x[:, :, F : F + 2] = 1.0
    x[:, :, F + 2 : FP] = 0.0
    return x


def kernel(**inputs) -> tuple:
    global last_results
    from concourse.bass_utils import run_bass_kernel_spmd

    f32 = np.float32
    f16 = np.float16
    Wfk = np.asarray(inputs["Wfk"], dtype=f32)
    bfk = np.asarray(inputs["bfk"], dtype=f32)
    Wbk = np.asarray(inputs["Wbk"], dtype=f32)
    bbk = np.asarray(inputs["bbk"], dtype=f32)
    Wr0 = np.asarray(inputs["Wr0"], dtype=f32)
    Wr1 = np.asarray(inputs["Wr1"], dtype=f32)
    wf_den = np.asarray(inputs["wf_den"], dtype=f32)
    wb_den = np.asarray(inputs["wb_den"], dtype=f32)
    i = int(np.asarray(inputs["i"]))
    num_utter = int(np.asarray(inputs["num_utter"]))

    x_f = _pack_x(np.asarray(inputs["K"]), np.asarray(inputs["front_k1"]))
    x_b = _pack_x(np.asarray(inputs["back_K"]), np.asarray(inputs["back_k2"]))

    adj_f = np.asarray(inputs["front_sdj_den"], dtype=f32)
    sm_f = np.asarray(inputs["front_s_mask"], dtype=f32)
    adj_b = np.asarray(inputs["back_sdj_den"], dtype=f32)
    sm_b = np.asarray(inputs["back_s_mask"], dtype=f32)

    def pack_mask(m):
        m = (NEGM * (1.0 - m)).astype(f16)
        m = m.reshape(NCORES, BL, 128, NT)
        return np.ascontiguousarray(m.transpose(0, 2, 1, 3))

    m0_f = pack_mask(adj_f * sm_f)
    m1_f = pack_mask(adj_f * (1.0 - sm_f))
    m0_b = pack_mask(adj_b * sm_b)
    m1_b = pack_mask(adj_b * (1.0 - sm_b))

    def fold_v(Wk, wden):
        v = np.zeros((FP,), f16)
        v[0:F] = (Wk.astype(np.float64) @ wden[D:].astype(np.float64)).astype(f16)
        return np.ascontiguousarray(np.broadcast_to(v, (128, FP)))

    v_f = fold_v(Wfk, wf_den)
    v_b = fold_v(Wbk, wb_den)
    A_f = np.vstack([Wfk, bfk[None, :]]).astype(np.float64)
    A_b = np.vstack([Wbk, bbk[None, :]]).astype(np.float64)
    def pack_g(G):
        gs = np.zeros((128, 7, D), f16)
        gs[:, 0:6, :] = G[0:F].reshape(6, 128, D).transpose(1, 0, 2)
        gs[0, 6, :] = G[F]
        return gs

    G0_f = pack_g((A_f @ Wr0.astype(np.float64)).astype(f16))
    G1_f = pack_g((A_f @ Wr1.astype(np.float64)).astype(f16))
    G0_b = pack_g((A_b @ Wr0.astype(np.float64)).astype(f16))
    G1_b = pack_g((A_b @ Wr1.astype(np.float64)).astype(f16))

    nc = _get_nc()

    in_maps = []
    for c in range(NCORES):
        s = slice(c * BL, (c + 1) * BL)
        in_maps.append(
            {
                "x_f": x_f[s],
                "x_b": x_b[s],
                "m0_f": m0_f[c],
                "m1_f": m1_f[c],
                "m0_b": m0_b[c],
                "m1_b": m1_b[c],
                "v_f": v_f,
                "v_b": v_b,
                "G0_f": G0_f,
                "G1_f": G1_f,
                "G0_b": G0_b,
                "G1_b": G1_b,
            }
        )

    trace = os.environ.get("KERNEL_TRACE", "0") == "1"
    res = run_bass_kernel_spmd(nc, in_maps, core_ids=list(range(NCORES)), trace=trace)
    last_results = res

    front = np.concatenate([r["out_f"] for r in res.results], axis=0)
    back = np.concatenate([r["out_b"] for r in res.results], axis=0)
    if i == 0:
        front = np.zeros((B, D), dtype=f32)
    if i == num_utter - 1:
        back = np.zeros((B, D), dtype=f32)
    return (front, back)
